# revision 9
# baseline (speedup 1.0000x reference)
"""HAN heterogeneous-graph-attention kernel for 8 Trainium2 NeuronCores.

SPMD over 8 cores, destination-sharded edges:
  - Each core owns dst slice [c*S,(c+1)*S) of each node type; edges bucketed
    into 128-dst windows, each window padded to a core-uniform tile count.
  - Projections are node-sharded; a combined rhs [W | W@A_src.. | W@A_dst..]
    yields h plus all attention dot-products in one matmul.  h (bf16) and
    per-node a_src (fp32, bit-packed into bf16 slots) form gather-table rows;
    tables are AllGathered so gathers are local.
  - Per edge tile (128 edges): one indirect-DMA row gather; one-hot P from a
    DVE compare vs iota; a_dst selected via P^T matmul; exp(leaky(...)) with
    -30000 masking for pads; segment sums via P^T @ [e | e*h] matmuls
    accumulated per window in PSUM; epilogue divides by segment sum + relu.
  - Semantic attention via DMA-transpose reads + tanh/accum, AllReduce of
    per-core colsums, tiny on-device softmax; fusion + LayerNorm + relu;
    layer-2 projection fused per tile via PE transposes; final classifier.
"""
import sys
sys.path.insert(0, "/opt/trn_rl_repo")
sys.path.insert(0, "/root/.axon_site")

import numpy as np
import ml_dtypes

bf16 = ml_dtypes.bfloat16
P = 128
CORES = 8
HEADS = 8

_cache = {}


# ---------------------------------------------------------------------------
# host-side planning
# ---------------------------------------------------------------------------

def _amat(att_vec, C):
    H, D = att_vec.shape
    A = np.zeros((C, H), np.float32)
    for hd in range(H):
        A[hd * D:(hd + 1) * D, hd] = att_vec[hd]
    return A


def _plan_edge_type(src, dst, n_src, S_dst, n_cores):
    W = -(-S_dst // P)
    order = np.argsort(dst, kind="stable")
    ds = dst[order].astype(np.int64)
    ss = src[order].astype(np.int64)
    bounds = np.searchsorted(ds, np.arange(n_cores + 1) * S_dst)
    counts = np.zeros((n_cores, W), np.int64)
    per_core = []
    for c in range(n_cores):
        lo, hi = bounds[c], bounds[c + 1]
        d_loc = ds[lo:hi] - c * S_dst
        w = d_loc >> 7
        counts[c] = np.bincount(w, minlength=W)
        per_core.append((d_loc, ss[lo:hi], w))
    tpw = np.maximum(-(-counts.max(axis=0) // P), 1).astype(np.int64)
    T = int(tpw.sum())
    tw0 = np.concatenate([[0], np.cumsum(tpw)])
    cores = []
    for c in range(n_cores):
        d_loc, s_loc, w = per_core[c]
        src_a = np.zeros((P, T), np.int32)
        rel_a = np.zeros((P, T), np.float32)
        msk_a = np.full((P, T), -30000.0, np.float32)
        wstart = np.concatenate([[0], np.cumsum(counts[c])])
        rank = np.arange(len(w)) - wstart[w]
        col = (tw0[w] + (rank >> 7)).astype(np.int64)
        row = (rank & 127).astype(np.int64)
        src_a[row, col] = s_loc
        rel_a[row, col] = d_loc & 127
        msk_a[row, col] = 0.0
        cores.append(dict(src=src_a, rel=rel_a.astype(bf16), msk=msk_a))
    return dict(tpw=[int(x) for x in tpw], T=T, W=W, cores=cores)


def _host_prep(inputs):
    x_addr = np.asarray(inputs["x_addr"], np.float32)
    x_tx = np.asarray(inputs["x_tx"], np.float32)
    N_ADDR, F_IN = x_addr.shape
    N_TX = x_tx.shape[0]
    HID = np.asarray(inputs["W1_addr"]).shape[1]
    OUT = np.asarray(inputs["W2_addr"]).shape[1]
    NCLS = np.asarray(inputs["lin_W"]).shape[1]
    S = {"addr": N_ADDR // CORES, "tx": N_TX // CORES}
    N = {"addr": N_ADDR, "tx": N_TX}

    ETS = [("a2t", "addr", "tx"), ("t2a", "tx", "addr"),
           ("a2a", "addr", "addr"), ("t2t", "tx", "tx")]
    L1_ETS = [0, 1, 2, 3]
    L2_ETS = [1, 2]
    src_of = {nt: [i for i, (_, st_, _) in enumerate(ETS) if st_ == nt] for nt in S}
    dst_of = {nt: [i for i, (_, _, dt_) in enumerate(ETS) if dt_ == nt] for nt in S}
    src2_of = {nt: [i for i in L2_ETS if ETS[i][1] == nt] for nt in S}
    dst2_of = {nt: [i for i in L2_ETS if ETS[i][2] == nt] for nt in S}

    f32 = lambda k: np.asarray(inputs[k], np.float32)
    att1_src, att1_dst = f32("att1_src"), f32("att1_dst")
    att2_src, att2_dst = f32("att2_src"), f32("att2_dst")

    def build_rhs(Wm, bm, att_s, att_d, srcs, dsts, C):
        cols, bcols = [Wm], [bm]
        for i in srcs:
            A = _amat(att_s[i], C); cols.append(Wm @ A); bcols.append(bm @ A)
        for i in dsts:
            A = _amat(att_d[i], C); cols.append(Wm @ A); bcols.append(bm @ A)
        return (np.concatenate(cols, 1).astype(bf16),
                np.concatenate(bcols, 0).astype(np.float32))

    rhs1, bias1, rhs2, bias2 = {}, {}, {}, {}
    for nt, Wk, bk in [("addr", "W1_addr", "b1_addr"), ("tx", "W1_tx", "b1_tx")]:
        rhs1[nt], bias1[nt] = build_rhs(f32(Wk), f32(bk), att1_src, att1_dst,
                                        src_of[nt], dst_of[nt], HID)
    for nt, Wk, bk in [("addr", "W2_addr", "b2_addr"), ("tx", "W2_tx", "b2_tx")]:
        rhs2[nt], bias2[nt] = build_rhs(f32(Wk), f32(bk), att2_src, att2_dst,
                                        src2_of[nt], dst2_of[nt], OUT)

    plans = []
    for i, (name, st_, dt_) in enumerate(ETS):
        plans.append(_plan_edge_type(
            np.asarray(inputs[f"{name}_src"]), np.asarray(inputs[f"{name}_dst"]),
            N[st_], S[dt_], CORES))

    xT = {"addr": np.ascontiguousarray(x_addr.T).astype(bf16),
          "tx": np.ascontiguousarray(x_tx.T).astype(bf16)}

    pad512 = lambda s: -(-s // 512) * 512

    static = dict(
        N_ADDR=N_ADDR, N_TX=N_TX, F_IN=F_IN, HID=HID, OUT=OUT, NCLS=NCLS,
        S=S, N=N, ETS=ETS, L1_ETS=L1_ETS, L2_ETS=L2_ETS,
        src_of=src_of, dst_of=dst_of, src2_of=src2_of, dst2_of=dst2_of,
        tpw=[p["tpw"] for p in plans], T=[p["T"] for p in plans],
        W=[p["W"] for p in plans],
        bias1_nz={nt: bool(np.any(bias1[nt])) for nt in S},
        bias2_nz={nt: bool(np.any(bias2[nt])) for nt in S},
        kb1_nz=bool(np.any(f32("k1_b"))), kb2_nz=bool(np.any(f32("k2_b"))),
        g1_triv=bool(np.all(f32("ln1_g") == 1) and not np.any(f32("ln1_b"))),
        g2_triv=bool(np.all(f32("ln2_g") == 1) and not np.any(f32("ln2_b"))),
        linb_nz=bool(np.any(f32("lin_b"))),
        Spad={nt: pad512(S[nt]) for nt in S},
    )

    # q columns with 1/N(dst type of pair) folded in
    q1 = f32("q1"); q2 = f32("q2")
    nh1, nh2 = HID // P, OUT // P
    q1c = np.zeros((P, 2 * nh1 * 0 + len(L1_ETS) * nh1), np.float32)
    for pi, et in enumerate(L1_ETS):
        scale = 1.0 / N[ETS[et][2]]
        for h in range(nh1):
            q1c[:, pi * nh1 + h] = q1[h * P:(h + 1) * P] * scale
    q2c = np.zeros((P, len(L2_ETS) * nh2), np.float32)
    for pi, et in enumerate(L2_ETS):
        scale = 1.0 / N[ETS[et][2]]
        for h in range(nh2):
            q2c[:, pi * nh2 + h] = q2[h * P:(h + 1) * P] * scale

    kb1c = np.ascontiguousarray(f32("k1_b").reshape(nh1, P).T)  # [128, nh1]
    kb2c = np.ascontiguousarray(f32("k2_b").reshape(nh2, P).T)

    shared = {
        "rhs1_addr": rhs1["addr"], "rhs1_tx": rhs1["tx"],
        "rhs2_addr": rhs2["addr"], "rhs2_tx": rhs2["tx"],
        "bias1_addr": bias1["addr"][None, :].astype(bf16),
        "bias1_tx": bias1["tx"][None, :].astype(bf16),
        "bias2_addr": bias2["addr"][None, :].astype(bf16),
        "bias2_tx": bias2["tx"][None, :].astype(bf16),
        "kW1": f32("k1_W").astype(bf16), "kW2": f32("k2_W").astype(bf16),
        "kb1": kb1c, "kb2": kb2c,
        "q1cols": q1c, "q2cols": q2c,
        "ln1_g": f32("ln1_g")[None, :], "ln1_b": f32("ln1_b")[None, :],
        "ln2_g": f32("ln2_g")[None, :], "ln2_b": f32("ln2_b")[None, :],
        "lin_W": f32("lin_W").astype(bf16),
        "lin_b": f32("lin_b")[None, :].astype(bf16),
    }
    in_maps = []
    for c in range(CORES):
        m = dict(shared)
        m["xT_addr"] = np.ascontiguousarray(
            xT["addr"][:, c * S["addr"]:(c + 1) * S["addr"]])
        m["xT_tx"] = np.ascontiguousarray(
            xT["tx"][:, c * S["tx"]:(c + 1) * S["tx"]])
        for i, pl in enumerate(plans):
            m[f"esrc{i}"] = pl["cores"][c]["src"]
            m[f"erel{i}"] = pl["cores"][c]["rel"]
            m[f"emsk{i}"] = pl["cores"][c]["msk"]
        in_maps.append(m)
    return static, in_maps


# ---------------------------------------------------------------------------
# device program
# ---------------------------------------------------------------------------

def _build_program(st):
    import contextlib
    import concourse.bass as bass
    import concourse.mybir as mybir
    import concourse.tile as tile
    from concourse import bacc
    from concourse.masks import make_identity

    dt = mybir.dt
    AF = mybir.ActivationFunctionType
    OP = mybir.AluOpType
    X = mybir.AxisListType.X

    S, N = st["S"], st["N"]
    HID, OUT, F_IN, NCLS = st["HID"], st["OUT"], st["F_IN"], st["NCLS"]
    ETS = st["ETS"]
    nc = bacc.Bacc("TRN2", target_bir_lowering=False, debug=False,
                   num_devices=CORES)

    io = {}
    def ein(name, shape, dty):
        io[name] = nc.dram_tensor(name, shape, dty, kind="ExternalInput")

    n_as1 = {nt: len(st["src_of"][nt]) for nt in S}
    n_ad1 = {nt: len(st["dst_of"][nt]) for nt in S}
    n_as2 = {nt: len(st["src2_of"][nt]) for nt in S}
    n_ad2 = {nt: len(st["dst2_of"][nt]) for nt in S}
    C1cols = {nt: HID + 8 * (n_as1[nt] + n_ad1[nt]) for nt in S}
    C2cols = {nt: OUT + 8 * (n_as2[nt] + n_ad2[nt]) for nt in S}
    ROW1 = {nt: HID + 16 * n_as1[nt] for nt in S}
    ROW2 = {nt: OUT + 16 * n_as2[nt] for nt in S}
    nh1, nh2 = HID // P, OUT // P

    ein("xT_addr", [F_IN, S["addr"]], dt.bfloat16)
    ein("xT_tx", [F_IN, S["tx"]], dt.bfloat16)
    for nt in S:
        ein(f"rhs1_{nt}", [F_IN, C1cols[nt]], dt.bfloat16)
        ein(f"bias1_{nt}", [1, C1cols[nt]], dt.bfloat16)
        ein(f"rhs2_{nt}", [HID, C2cols[nt]], dt.bfloat16)
        ein(f"bias2_{nt}", [1, C2cols[nt]], dt.bfloat16)
    ein("kW1", [HID, HID], dt.bfloat16)
    ein("kW2", [OUT, OUT], dt.bfloat16)
    ein("kb1", [P, nh1], dt.float32)
    ein("kb2", [P, nh2], dt.float32)
    ein("q1cols", [P, len(st["L1_ETS"]) * nh1], dt.float32)
    ein("q2cols", [P, len(st["L2_ETS"]) * nh2], dt.float32)
    ein("ln1_g", [1, HID], dt.float32); ein("ln1_b", [1, HID], dt.float32)
    ein("ln2_g", [1, OUT], dt.float32); ein("ln2_b", [1, OUT], dt.float32)
    ein("lin_W", [OUT, NCLS], dt.bfloat16)
    ein("lin_b", [1, NCLS], dt.bfloat16)
    for i in range(4):
        ein(f"esrc{i}", [P, st["T"][i]], dt.int32)
        ein(f"erel{i}", [P, st["T"][i]], dt.bfloat16)
        ein(f"emsk{i}", [P, st["T"][i]], dt.float32)
    out_t = nc.dram_tensor("out", [S["addr"], NCLS], dt.float32,
                           kind="ExternalOutput")
    RG = [list(range(CORES))]

    with tile.TileContext(nc) as tc, contextlib.ExitStack() as ctx:
        dram = ctx.enter_context(tc.tile_pool(name="dram", bufs=1, space="DRAM"))
        const = ctx.enter_context(tc.tile_pool(name="const", bufs=1))

        # ---- persistent DRAM ----
        Wn = {nt: -(-S[nt] // P) for nt in S}
        tab1_loc = {nt: dram.tile([S[nt], ROW1[nt]], dt.bfloat16,
                                  name=f"tab1loc_{nt}") for nt in S}
        tab1 = {nt: dram.tile([N[nt], ROW1[nt]], dt.bfloat16,
                              addr_space="Shared", name=f"tab1_{nt}") for nt in S}
        tab2_loc = {nt: dram.tile([S[nt], ROW2[nt]], dt.bfloat16,
                                  name=f"tab2loc_{nt}") for nt in S}
        tab2 = {nt: dram.tile([N[nt], ROW2[nt]], dt.bfloat16,
                              addr_space="Shared", name=f"tab2_{nt}") for nt in S}
        ad1 = {nt: dram.tile([Wn[nt] * P, 16], dt.float32, name=f"ad1_{nt}")
               for nt in S}
        ad2 = {"addr": dram.tile([Wn["addr"] * P, 16], dt.float32, name="ad2_addr")}
        o1 = {i: dram.tile([st["Spad"][ETS[i][2]], HID], dt.bfloat16,
                           name=f"o1_{i}") for i in st["L1_ETS"]}
        o2 = {i: dram.tile([st["Spad"]["addr"], OUT], dt.bfloat16,
                           name=f"o2_{i}") for i in st["L2_ETS"]}
        npair1 = len(st["L1_ETS"]) * nh1
        npair2 = len(st["L2_ETS"]) * nh2
        sc1_in = dram.tile([P, npair1], dt.float32, name="sc1_in")
        sc1_out = dram.tile([P, npair1], dt.float32, addr_space="Shared",
                            name="sc1_out")
        sc2_in = dram.tile([P, npair2], dt.float32, name="sc2_in")
        sc2_out = dram.tile([P, npair2], dt.float32, addr_space="Shared",
                            name="sc2_out")

        # ---- constants ----
        iota_row = const.tile([P, P], dt.bfloat16)
        nc.gpsimd.iota(iota_row[:], pattern=[[1, P]], base=0,
                       channel_multiplier=0, allow_small_or_imprecise_dtypes=True)
        ident = const.tile([P, P], dt.bfloat16)
        make_identity(nc, ident[:])
        ones_row = const.tile([1, P], dt.bfloat16)
        nc.vector.memset(ones_row[:], 1.0)
        ones_row_f = const.tile([1, P], dt.float32)
        nc.vector.memset(ones_row_f[:], 1.0)
        ones_col_f = const.tile([P, 1], dt.float32)
        nc.vector.memset(ones_col_f[:], 1.0)
        eps_ln = const.tile([P, 1], dt.float32)
        nc.vector.memset(eps_ln[:], 1e-5)
        zrow = const.tile([P, 640], dt.bfloat16)
        nc.vector.memset(zrow[:], 0.0)
        zrow_f = const.tile([P, 16], dt.float32)
        nc.vector.memset(zrow_f[:], 0.0)

        for nt in S:
            padn = Wn[nt] * P - S[nt]
            if padn:
                nc.sync.dma_start(ad1[nt][S[nt]:, :], zrow_f[:padn, :])
                if nt in ad2:
                    nc.sync.dma_start(ad2[nt][S[nt]:, :], zrow_f[:padn, :])
        for i, o in o1.items():
            Sr = S[ETS[i][2]]
            padn = st["Spad"][ETS[i][2]] - Sr
            for r0 in range(0, padn, P):
                rr = min(P, padn - r0)
                nc.sync.dma_start(o[Sr + r0:Sr + r0 + rr, :], zrow[:rr, :HID])
        for i, o in o2.items():
            Sr = S["addr"]
            padn = st["Spad"]["addr"] - Sr
            for r0 in range(0, padn, P):
                rr = min(P, padn - r0)
                nc.sync.dma_start(o[Sr + r0:Sr + r0 + rr, :], zrow[:rr, :OUT])

        # ---- weights in SBUF ----
        sb = {}
        def load_blocks(key, R, C):
            ts = []
            for k in range(-(-R // P)):
                r = min(P, R - k * P)
                t = const.tile([r, C], dt.bfloat16, name=f"sb_{key}_{k}")
                nc.sync.dma_start(t[:], io[key][k * P:k * P + r, :])
                ts.append(t)
            return ts
        for nt in S:
            sb[f"rhs1_{nt}"] = load_blocks(f"rhs1_{nt}", F_IN, C1cols[nt])
            sb[f"rhs2_{nt}"] = load_blocks(f"rhs2_{nt}", HID, C2cols[nt])
            for b in (f"bias1_{nt}", f"bias2_{nt}"):
                C = C1cols[nt] if b.startswith("bias1") else C2cols[nt]
                t = const.tile([1, C], dt.bfloat16, name=f"sb_{b}")
                nc.sync.dma_start(t[:], io[b][:])
                sb[b] = t
        sb["kW1"] = load_blocks("kW1", HID, HID)
        sb["kW2"] = load_blocks("kW2", OUT, OUT)
        sb["lin_W"] = load_blocks("lin_W", OUT, NCLS)
        t = const.tile([1, NCLS], dt.bfloat16, name="sb_linb")
        nc.sync.dma_start(t[:], io["lin_b"][:])
        sb["lin_b"] = t
        for k, sh in [("kb1", [P, nh1]), ("kb2", [P, nh2]),
                      ("q1cols", [P, npair1]), ("q2cols", [P, npair2])]:
            t = const.tile(sh, dt.float32, name=f"sb_{k}")
            nc.sync.dma_start(t[:], io[k][:])
            sb[k] = t

        ln_bc = {}
        for L, C, triv in [(1, HID, st["g1_triv"]), (2, OUT, st["g2_triv"])]:
            if triv:
                continue
            with tc.tile_pool(name=f"lnp{L}", bufs=2, space="PSUM") as lnp:
                for suffix in ("g", "b"):
                    k = f"ln{L}_{suffix}"
                    row = const.tile([1, C], dt.float32, name=f"row_{k}")
                    nc.sync.dma_start(row[:], io[k][:])
                    ps = lnp.tile([P, C], dt.float32, space="PSUM", name="lnps")
                    nc.tensor.matmul(ps[:], ones_row_f[:], row[:],
                                     start=True, stop=True)
                    t = const.tile([P, C], dt.float32, name=f"bc_{k}")
                    nc.vector.tensor_copy(t[:], ps[:])
                    ln_bc[k] = t

        meta = {}
        for i in range(4):
            for pre, dty in [("esrc", dt.int32), ("erel", dt.bfloat16),
                             ("emsk", dt.float32)]:
                t = const.tile([P, st["T"][i]], dty, name=f"{pre}{i}")
                nc.sync.dma_start(t[:], io[f"{pre}{i}"][:])
                meta[f"{pre}{i}"] = t

        # =========================================================
        def proj_pass(L, nt, get_lhsT, Ccols, Crow, Cdim, tab_loc_t, ad_t,
                      n_as, n_ad, bias_nz, extra_cb=None):
            n_tiles = -(-S[nt] // P)
            with (
                tc.tile_pool(name=f"pp{L}{nt}", bufs=3) as pp,
                tc.tile_pool(name=f"pps{L}{nt}", bufs=4, space="PSUM") as pps,
            ):
                SLAB = 8
                out_slab = ad_slab = None
                for i in range(n_tiles):
                    rows = min(P, S[nt] - i * P)
                    if i % SLAB == 0:
                        out_slab = pp.tile([P, SLAB, Crow], dt.bfloat16,
                                           name="oslab")
                        ad_slab = (pp.tile([P, SLAB, 16], dt.float32,
                                           name="adslab") if n_ad else None)
                    j = i % SLAB
                    ps = pps.tile([P, Ccols], dt.float32, space="PSUM",
                                  name="projps")
                    blocks = get_lhsT(i, pp, pps)
                    nb = len(blocks) + (1 if bias_nz else 0)
                    for bi, lhsT in enumerate(blocks):
                        nc.tensor.matmul(ps[:rows, :], lhsT,
                                         sb[f"rhs{L}_{nt}"][bi][:],
                                         start=(bi == 0), stop=(bi == nb - 1))
                    if bias_nz:
                        nc.tensor.matmul(ps[:rows, :], ones_row[:, :rows],
                                         sb[f"bias{L}_{nt}"][:],
                                         start=False, stop=True)
                    nc.any.tensor_copy(out_slab[:rows, j, 0:Cdim],
                                       ps[:rows, 0:Cdim])
                    if n_as:
                        nc.vector.tensor_copy(
                            out_slab[:rows, j, Cdim:Cdim + 16 * n_as]
                            .bitcast(dt.float32),
                            ps[:rows, Cdim:Cdim + 8 * n_as])
                    if n_ad:
                        nc.vector.tensor_copy(
                            ad_slab[:rows, j, 0:8 * n_ad],
                            ps[:rows, Cdim + 8 * n_as:Cdim + 8 * (n_as + n_ad)])
                        if n_ad == 1:
                            nc.vector.memset(ad_slab[:rows, j, 8:16], 0.0)
                    if j == SLAB - 1 or i == n_tiles - 1:
                        i0 = (i // SLAB) * SLAB
                        k = i - i0 + 1
                        kf = k - (0 if rows == P else 1)
                        if kf:
                            nc.sync.dma_start(
                                tab_loc_t[i0 * P:i0 * P + kf * P, :]
                                .rearrange("(k p) r -> p k r", p=P),
                                out_slab[:, 0:kf, :])
                            if n_ad:
                                nc.sync.dma_start(
                                    ad_t[i0 * P:i0 * P + kf * P, :]
                                    .rearrange("(k p) r -> p k r", p=P),
                                    ad_slab[:, 0:kf, :])
                        if rows < P:
                            nc.sync.dma_start(tab_loc_t[i * P:i * P + rows, :],
                                              out_slab[:rows, k - 1, :])
                            if n_ad:
                                nc.sync.dma_start(ad_t[i * P:i * P + rows, :],
                                                  ad_slab[:rows, k - 1, :])

        # ---- phase 1: layer-1 projections ----
        with tc.tile_pool(name="xts", bufs=3) as xts:
            for nt in S:
                state = {}

                def get_lhsT1(i, pp, pps, nt=nt, state=state):
                    GS = 8
                    g = i // GS
                    if state.get("g") != g:
                        cols = min(GS * P, S[nt] - g * GS * P)
                        tl = xts.tile([F_IN, GS * P], dt.bfloat16,
                                      name=f"xt_{nt}")
                        nc.sync.dma_start(
                            tl[:, :cols],
                            io[f"xT_{nt}"][:, g * GS * P:g * GS * P + cols])
                        state["g"] = g
                        state["t"] = tl
                    off = (i % GS) * P
                    cols = min(P, S[nt] - i * P)
                    return [state["t"][:, off:off + cols]]

                proj_pass(1, nt, get_lhsT1, C1cols[nt], ROW1[nt], HID,
                          tab1_loc[nt], ad1[nt], n_as1[nt], n_ad1[nt],
                          st["bias1_nz"][nt])
        for nt in S:
            nc.gpsimd.collective_compute(
                "AllGather", OP.bypass, replica_groups=RG,
                ins=[tab1_loc[nt][:]], outs=[tab1[nt][0:N[nt], :]])

        # =========================================================
        def edge_pass(L, ets, tabs, ROWt, ads, os_, Cdim, asrc_idx, adcol_idx):
            D = Cdim // HEADS
            BMAX = 12
            for et in ets:
                _, stp, dtp = ETS[et]
                tpw = st["tpw"][et]
                Srows = S[dtp]
                tw0 = np.concatenate([[0], np.cumsum(tpw)]).astype(int)
                batches, cur, curB = [], [], 0
                for w in range(st["W"][et]):
                    if cur and (curB + tpw[w] > BMAX or len(cur) >= 4):
                        batches.append(cur); cur, curB = [], 0
                    cur.append(w); curB += tpw[w]
                if cur:
                    batches.append(cur)
                aoff = asrc_idx[et] * 16
                acol = adcol_idx[et] * 8
                with (
                    tc.tile_pool(name=f"eg{L}{et}", bufs=3) as eg,
                    tc.tile_pool(name=f"eps{L}{et}", bufs=4, space="PSUM") as eps1,
                    tc.tile_pool(name=f"ep2{L}{et}", bufs=2, space="PSUM") as eps2,
                ):
                    for wins in batches:
                        t0, t1 = int(tw0[wins[0]]), int(tw0[wins[-1] + 1])
                        B = t1 - t0
                        w0, nw = wins[0], len(wins)
                        G = eg.tile([P, B, ROWt[stp]], dt.bfloat16, name="G")
                        for b in range(B):
                            nc.gpsimd.indirect_dma_start(
                                out=G[:, b, :], out_offset=None,
                                in_=tabs[stp][:],
                                in_offset=bass.IndirectOffsetOnAxis(
                                    ap=meta[f"esrc{et}"][:, t0 + b:t0 + b + 1],
                                    axis=0))
                        adw = eg.tile([P, nw, 16], dt.float32, name="adw")
                        nc.sync.dma_start(
                            adw[:], ads[dtp][w0 * P:(w0 + nw) * P, :]
                            .rearrange("(k p) r -> p k r", p=P))
                        adw_bf = eg.tile([P, nw, 16], dt.bfloat16, name="adwbf")
                        nc.vector.tensor_copy(adw_bf[:], adw[:])
                        Pm = eg.tile([P, B, P], dt.bfloat16, name="Pm")
                        nc.vector.tensor_tensor(
                            out=Pm[:],
                            in0=meta[f"erel{et}"][:, t0:t1, None]
                            .to_broadcast([P, B, P]),
                            in1=iota_row[:, None, :].to_broadcast([P, B, P]),
                            op=OP.is_equal)
                        ad_ps = eps2.tile([P, B, 8], dt.float32, space="PSUM",
                                          name="adps")
                        for bw, w in enumerate(wins):
                            for b in range(tw0[w] - t0, tw0[w + 1] - t0):
                                pt_ps = eps2.tile([P, P], dt.bfloat16,
                                                  space="PSUM", name="ptps")
                                nc.tensor.transpose(pt_ps[:], Pm[:, b, :],
                                                    ident[:])
                                pt_sb = eg.tile([P, P], dt.bfloat16, name="ptsb")
                                nc.vector.tensor_copy(pt_sb[:], pt_ps[:])
                                nc.tensor.matmul(
                                    ad_ps[:, b, :], pt_sb[:],
                                    adw_bf[:, bw, acol:acol + 8],
                                    start=True, stop=True,
                                    skip_group_check=True)
                        alpha = eg.tile([P, B, 8], dt.float32, name="alpha")
                        nc.vector.tensor_tensor(
                            out=alpha[:],
                            in0=G[:, :, Cdim + aoff:Cdim + aoff + 16]
                            .bitcast(dt.float32),
                            in1=ad_ps[:], op=OP.add)
                        nc.vector.tensor_tensor(
                            out=alpha[:], in0=alpha[:],
                            in1=meta[f"emsk{et}"][:, t0:t1, None]
                            .to_broadcast([P, B, 8]), op=OP.add)
                        lr = eg.tile([P, B, 8], dt.float32, name="lr")
                        nc.vector.scalar_tensor_tensor(
                            lr[:], alpha[:], 0.2, alpha[:],
                            op0=OP.mult, op1=OP.max)
                        msg = eg.tile([P, B, 8 + Cdim], dt.bfloat16, name="msg")
                        nc.scalar.activation(msg[:, :, 0:8], lr[:], AF.Exp)
                        nc.vector.tensor_tensor(
                            out=msg[:, :, 8:8 + Cdim]
                            .rearrange("p b (h d) -> p b h d", h=HEADS),
                            in0=G[:, :, 0:Cdim]
                            .rearrange("p b (h d) -> p b h d", h=HEADS),
                            in1=msg[:, :, 0:8][:, :, :, None]
                            .to_broadcast([P, B, HEADS, D]),
                            op=OP.mult)
                        o_slab = eg.tile([P, nw, Cdim], dt.bfloat16,
                                         name="oslabE")
                        for bw, w in enumerate(wins):
                            ps = eps1.tile([P, 8 + Cdim], dt.float32,
                                           space="PSUM", name="aggps")
                            bs = list(range(tw0[w] - t0, tw0[w + 1] - t0))
                            for k, b in enumerate(bs):
                                nc.tensor.matmul(ps[:], Pm[:, b, :],
                                                 msg[:, b, :],
                                                 start=(k == 0),
                                                 stop=(k == len(bs) - 1))
                            srec = eg.tile([P, 8], dt.float32, name="srec")
                            nc.vector.tensor_scalar(srec[:], ps[:, 0:8],
                                                    1e-16, None, op0=OP.add)
                            nc.vector.reciprocal(srec[:], srec[:])
                            nc.vector.tensor_tensor(
                                out=o_slab[:, bw, :]
                                .rearrange("p (h d) -> p h d", h=HEADS),
                                in0=ps[:, 8:8 + Cdim]
                                .rearrange("p (h d) -> p h d", h=HEADS),
                                in1=srec[:, :, None]
                                .to_broadcast([P, HEADS, D]),
                                op=OP.mult)
                        nc.vector.tensor_scalar(o_slab[:], o_slab[:], 0.0,
                                                None, op0=OP.max)
                        lastw = wins[-1]
                        full = nw - (1 if (lastw + 1) * P > Srows else 0)
                        if full:
                            nc.sync.dma_start(
                                os_[et][w0 * P:(w0 + full) * P, :]
                                .rearrange("(k p) r -> p k r", p=P),
                                o_slab[:, 0:full, :])
                        if full < nw:
                            rr = Srows - lastw * P
                            nc.sync.dma_start(os_[et][lastw * P:Srows, :],
                                              o_slab[:rr, nw - 1, :])

        # =========================================================
        def score_pass(L, ets, os_, Cdim, kWk, kbk, qk, kb_nz, sc_in, sc_out):
            nh = Cdim // P
            acc = const.tile([P, len(ets) * nh], dt.float32, name=f"scacc{L}")
            nc.vector.memset(acc[:], 0.0)
            with (
                tc.tile_pool(name=f"sp{L}", bufs=3) as sp,
                tc.tile_pool(name=f"sps{L}", bufs=4, space="PSUM") as sps,
            ):
                for pi, et in enumerate(ets):
                    dtp = ETS[et][2]
                    Srp = st["Spad"][dtp]
                    for n0 in range(0, Srp, 512):
                        oTs = []
                        for k in range(nh):
                            oT = sp.tile([P, 512], dt.bfloat16, name="oT")
                            nc.sync.dma_start_transpose(
                                out=oT[:],
                                in_=os_[et][n0:n0 + 512, k * P:(k + 1) * P])
                            oTs.append(oT)
                        for h in range(nh):
                            zps = sps.tile([P, 512], dt.float32, space="PSUM",
                                           name="zps")
                            for k in range(nh):
                                nc.tensor.matmul(
                                    zps[:], sb[kWk][k][:, h * P:(h + 1) * P],
                                    oTs[k][:], start=(k == 0),
                                    stop=(k == nh - 1))
                            th = sp.tile([P, 512], dt.float32, name="th")
                            chs = sp.tile([P, 1], dt.float32, name="chs")
                            if kb_nz:
                                nc.scalar.activation(th[:], zps[:], AF.Tanh,
                                                     bias=sb[kbk][:, h:h + 1],
                                                     accum_out=chs[:])
                            else:
                                nc.scalar.activation(th[:], zps[:], AF.Tanh,
                                                     accum_out=chs[:])
                            nc.vector.tensor_tensor(
                                acc[:, pi * nh + h:pi * nh + h + 1],
                                acc[:, pi * nh + h:pi * nh + h + 1],
                                chs[:], op=OP.add)
                    if kb_nz:
                        npad = Srp - S[dtp]
                        if npad:
                            for h in range(nh):
                                kbt = sp.tile([P, 1], dt.float32, name="kbt")
                                nc.scalar.activation(kbt[:],
                                                     sb[kbk][:, h:h + 1],
                                                     AF.Tanh)
                                nc.vector.scalar_tensor_tensor(
                                    acc[:, pi * nh + h:pi * nh + h + 1],
                                    kbt[:], -float(npad),
                                    acc[:, pi * nh + h:pi * nh + h + 1],
                                    op0=OP.mult, op1=OP.add)
                qprod = sp.tile([P, len(ets) * nh], dt.float32, name="qprod")
                nc.vector.tensor_tensor(qprod[:], acc[:], sb[qk][:], op=OP.mult)
                nc.sync.dma_start(sc_in[:], qprod[:])
            nc.gpsimd.collective_compute(
                "AllReduce", OP.add, replica_groups=RG,
                ins=[sc_in[:]], outs=[sc_out[:]])

        def softmax_weights(L, ets, groups, sc_out, Cdim):
            """groups: dict nt -> (pi_a, pi_b). Returns w_bc [128, npair] f32."""
            nh = Cdim // P
            npair = len(ets) * nh
            with (
                tc.tile_pool(name=f"sw{L}", bufs=1) as sw,
                tc.tile_pool(name=f"swp{L}", bufs=2, space="PSUM") as swp,
            ):
                scs = sw.tile([P, npair], dt.float32, name="scs")
                nc.sync.dma_start(scs[:], sc_out[:])
                rps = swp.tile([1, npair], dt.float32, space="PSUM", name="rps")
                nc.tensor.matmul(rps[:], ones_col_f[:], scs[:],
                                 start=True, stop=True)
                srow = sw.tile([1, len(ets)], dt.float32, name="srow")
                if nh > 1:
                    nc.vector.tensor_reduce(
                        srow[:], rps[:].rearrange("o (e h) -> o e h", h=nh),
                        axis=X, op=OP.add)
                else:
                    nc.vector.tensor_copy(srow[:], rps[:])
                wrow = sw.tile([1, len(ets)], dt.float32, name="wrow")
                for nt, (pa, pb) in groups.items():
                    m = sw.tile([1, 1], dt.float32, name="m")
                    nc.vector.tensor_tensor(m[:], srow[:, pa:pa + 1],
                                            srow[:, pb:pb + 1], op=OP.max)
                    ea = sw.tile([1, 1], dt.float32, name="ea")
                    eb = sw.tile([1, 1], dt.float32, name="eb")
                    da = sw.tile([1, 1], dt.float32, name="da")
                    db = sw.tile([1, 1], dt.float32, name="db")
                    nc.vector.tensor_tensor(da[:], srow[:, pa:pa + 1], m[:],
                                            op=OP.subtract)
                    nc.vector.tensor_tensor(db[:], srow[:, pb:pb + 1], m[:],
                                            op=OP.subtract)
                    nc.scalar.activation(ea[:], da[:], AF.Exp)
                    nc.scalar.activation(eb[:], db[:], AF.Exp)
                    ssum = sw.tile([1, 1], dt.float32, name="ssum")
                    nc.vector.tensor_tensor(ssum[:], ea[:], eb[:], op=OP.add)
                    nc.vector.reciprocal(ssum[:], ssum[:])
                    nc.vector.tensor_tensor(wrow[:, pa:pa + 1], ea[:], ssum[:],
                                            op=OP.mult)
                    nc.vector.tensor_tensor(wrow[:, pb:pb + 1], eb[:], ssum[:],
                                            op=OP.mult)
                wps = swp.tile([P, len(ets)], dt.float32, space="PSUM",
                               name="wps")
                nc.tensor.matmul(wps[:], ones_row_f[:], wrow[:],
                                 start=True, stop=True)
                w_bc = const.tile([P, len(ets)], dt.float32, name=f"wbc{L}")
                nc.vector.tensor_copy(w_bc[:], wps[:])
            return w_bc

        def fuse_ln(pp, pps, i, nt, oa, ob, w_bc, pa, pb, Cdim, g_triv, L):
            """Load o tiles, fuse, LN, relu -> bf16 tile [rows, Cdim]."""
            rows = min(P, S[nt] - i * P)
            ta = pp.tile([P, Cdim], dt.bfloat16, name="fl_oa")
            tb = pp.tile([P, Cdim], dt.bfloat16, name="fl_ob")
            nc.sync.dma_start(ta[:rows], oa[i * P:i * P + rows, :])
            nc.sync.dma_start(tb[:rows], ob[i * P:i * P + rows, :])
            fused = pp.tile([P, Cdim], dt.float32, name="fl_fused")
            nc.vector.tensor_scalar(fused[:rows], ta[:rows],
                                    w_bc[:rows, pa:pa + 1], None, op0=OP.mult)
            nc.vector.scalar_tensor_tensor(fused[:rows], tb[:rows],
                                           w_bc[:rows, pb:pb + 1], fused[:rows],
                                           op0=OP.mult, op1=OP.add)
            srow = pp.tile([P, 1], dt.float32, name="fl_srow")
            nc.vector.tensor_reduce(srow[:rows], fused[:rows], axis=X, op=OP.add)
            mu = pp.tile([P, 1], dt.float32, name="fl_mu")
            nc.vector.tensor_scalar(mu[:rows], srow[:rows], 1.0 / Cdim, None,
                                    op0=OP.mult)
            sq = pp.tile([P, Cdim], dt.float32, name="fl_sq")
            ssq = pp.tile([P, 1], dt.float32, name="fl_ssq")
            nc.scalar.activation(sq[:rows], fused[:rows], AF.Square,
                                 accum_out=ssq[:rows])
            musq = pp.tile([P, 1], dt.float32, name="fl_musq")
            nc.vector.tensor_tensor(musq[:rows], mu[:rows], mu[:rows],
                                    op=OP.mult)
            var = pp.tile([P, 1], dt.float32, name="fl_var")
            nc.vector.scalar_tensor_tensor(var[:rows], ssq[:rows], 1.0 / Cdim,
                                           musq[:rows], op0=OP.mult,
                                           op1=OP.subtract)
            nc.vector.tensor_scalar(var[:rows], var[:rows], 0.0, None,
                                    op0=OP.max)
            stdt = pp.tile([P, 1], dt.float32, name="fl_std")
            nc.scalar.activation(stdt[:rows], var[:rows], AF.Sqrt,
                                 bias=eps_ln[:rows])
            rstd = pp.tile([P, 1], dt.float32, name="fl_rstd")
            nc.vector.reciprocal(rstd[:rows], stdt[:rows])
            xn = pp.tile([P, Cdim], dt.float32, name="fl_xn")
            nc.vector.tensor_scalar(xn[:rows], fused[:rows], mu[:rows],
                                    rstd[:rows], op0=OP.subtract, op1=OP.mult)
            if not g_triv:
                nc.vector.tensor_tensor(xn[:rows], xn[:rows],
                                        ln_bc[f"ln{L}_g"][:rows], op=OP.mult)
                nc.vector.tensor_tensor(xn[:rows], xn[:rows],
                                        ln_bc[f"ln{L}_b"][:rows], op=OP.add)
            h_bf = pp.tile([P, Cdim], dt.bfloat16, name="fl_hbf")
            nc.vector.tensor_scalar(h_bf[:rows], xn[:rows], 0.0, None,
                                    op0=OP.max)
            return h_bf

        # ---- layer 1 edges ----
        asrc_idx1 = {et: st["src_of"][ETS[et][1]].index(et)
                     for et in st["L1_ETS"]}
        adcol_idx1 = {et: st["dst_of"][ETS[et][2]].index(et)
                      for et in st["L1_ETS"]}
        edge_pass(1, st["L1_ETS"], tab1, ROW1, ad1, o1, HID,
                  asrc_idx1, adcol_idx1)

        # ---- layer 1 score + weights ----
        score_pass(1, st["L1_ETS"], o1, HID, "kW1", "kb1", "q1cols",
                   st["kb1_nz"], sc1_in, sc1_out)
        groups1 = {}
        for nt in S:
            pair = st["dst_of"][nt]
            groups1[nt] = (st["L1_ETS"].index(pair[0]),
                           st["L1_ETS"].index(pair[1]))
        w1_bc = softmax_weights(1, st["L1_ETS"], groups1, sc1_out, HID)

        # ---- layer 1 fusion + LN + layer-2 projection ----
        for nt in S:
            pa, pb = groups1[nt]
            eta, etb = st["dst_of"][nt]

            def get_lhsT2(i, pp, pps, nt=nt, pa=pa, pb=pb, eta=eta, etb=etb):
                h_bf = fuse_ln(pp, pps, i, nt, o1[eta], o1[etb], w1_bc,
                               pa, pb, HID, st["g1_triv"], 1)
                rows = min(P, S[nt] - i * P)
                blocks = []
                for k in range(HID // P):
                    tp_ps = pps.tile([P, P], dt.bfloat16, space="PSUM",
                                     name="tpps", bufs=2)
                    nc.tensor.transpose(tp_ps[:, :rows],
                                        h_bf[:rows, k * P:(k + 1) * P],
                                        ident[:rows, :rows])
                    tsb = pp.tile([P, P], dt.bfloat16, name=f"h1T_{k}")
                    nc.vector.tensor_copy(tsb[:, :rows], tp_ps[:, :rows])
                    blocks.append(tsb[:, :rows])
                return blocks

            proj_pass(2, nt, get_lhsT2, C2cols[nt], ROW2[nt], OUT,
                      tab2_loc[nt], ad2.get(nt), n_as2[nt], n_ad2[nt],
                      st["bias2_nz"][nt])
        for nt in S:
            nc.gpsimd.collective_compute(
                "AllGather", OP.bypass, replica_groups=RG,
                ins=[tab2_loc[nt][:]], outs=[tab2[nt][0:N[nt], :]])

        # ---- layer 2 edges ----
        asrc_idx2 = {et: st["src2_of"][ETS[et][1]].index(et)
                     for et in st["L2_ETS"]}
        adcol_idx2 = {et: st["dst2_of"][ETS[et][2]].index(et)
                      for et in st["L2_ETS"]}
        edge_pass(2, st["L2_ETS"], tab2, ROW2, ad2, o2, OUT,
                  asrc_idx2, adcol_idx2)

        # ---- layer 2 score + weights ----
        score_pass(2, st["L2_ETS"], o2, OUT, "kW2", "kb2", "q2cols",
                   st["kb2_nz"], sc2_in, sc2_out)
        pair2 = st["dst2_of"]["addr"]
        groups2 = {"addr": (st["L2_ETS"].index(pair2[0]),
                            st["L2_ETS"].index(pair2[1]))}
        w2_bc = softmax_weights(2, st["L2_ETS"], groups2, sc2_out, OUT)

        # ---- layer 2 fusion + classifier ----
        pa2, pb2 = groups2["addr"]
        eta2, etb2 = pair2
        n_tiles = -(-S["addr"] // P)
        with (
            tc.tile_pool(name="cls", bufs=3) as cls,
            tc.tile_pool(name="clsp", bufs=4, space="PSUM") as clsp,
        ):
            SLAB = 8
            out_slab = None
            for i in range(n_tiles):
                rows = min(P, S["addr"] - i * P)
                if i % SLAB == 0:
                    out_slab = cls.tile([P, SLAB, NCLS], dt.float32,
                                        name="cl_oslab")
                j = i % SLAB
                h_bf = fuse_ln(cls, clsp, i, "addr", o2[eta2], o2[etb2],
                               w2_bc, pa2, pb2, OUT, st["g2_triv"], 2)
                ps = clsp.tile([P, NCLS], dt.float32, space="PSUM",
                               name="cl_ps")
                nbk = OUT // P
                nb = nbk + (1 if st["linb_nz"] else 0)
                for k in range(nbk):
                    tp_ps = clsp.tile([P, P], dt.bfloat16, space="PSUM",
                                      name="cl_tp", bufs=2)
                    nc.tensor.transpose(tp_ps[:, :rows],
                                        h_bf[:rows, k * P:(k + 1) * P],
                                        ident[:rows, :rows])
                    tsb = cls.tile([P, P], dt.bfloat16, name="cl_h2T")
                    nc.vector.tensor_copy(tsb[:, :rows], tp_ps[:, :rows])
                    nc.tensor.matmul(ps[:rows], tsb[:, :rows],
                                     sb["lin_W"][k][:],
                                     start=(k == 0), stop=(k == nb - 1))
                if st["linb_nz"]:
                    nc.tensor.matmul(ps[:rows], ones_row[:, :rows],
                                     sb["lin_b"][:], start=False, stop=True)
                nc.vector.tensor_copy(out_slab[:rows, j, :], ps[:rows])
                if j == SLAB - 1 or i == n_tiles - 1:
                    i0 = (i // SLAB) * SLAB
                    k = i - i0 + 1
                    kf = k - (0 if rows == P else 1)
                    if kf:
                        nc.sync.dma_start(
                            out_t[i0 * P:i0 * P + kf * P, :]
                            .rearrange("(k p) r -> p k r", p=P),
                            out_slab[:, 0:kf, :])
                    if rows < P:
                        nc.sync.dma_start(out_t[i * P:i * P + rows, :],
                                          out_slab[:rows, k - 1, :])

    nc.compile()
    return nc


# ---------------------------------------------------------------------------
# runner (cached compile via persistent jitted callable)
# ---------------------------------------------------------------------------

def _make_runner(nc, n_cores):
    import jax
    from jax.sharding import Mesh, PartitionSpec
    from jax.experimental.shard_map import shard_map
    import concourse.mybir as mybir
    import concourse.bass2jax as b2j

    b2j.install_neuronx_cc_hook()
    partition_name = nc.partition_id_tensor.name if nc.partition_id_tensor else None
    in_names, out_names, out_avals, zero_outs = [], [], [], []
    for alloc in nc.m.functions[0].allocations:
        if not isinstance(alloc, mybir.MemoryLocationSet):
            continue
        name = alloc.memorylocations[0].name
        if alloc.kind == "ExternalInput":
            if name != partition_name:
                in_names.append(name)
        elif alloc.kind == "ExternalOutput":
            shape = tuple(alloc.tensor_shape)
            dtype = mybir.dt.np(alloc.dtype)
            out_names.append(name)
            out_avals.append(jax.core.ShapedArray(shape, dtype))
            zero_outs.append(np.zeros(shape, dtype))
    n_params = len(in_names)
    n_outs = len(out_avals)
    all_in_names = list(in_names) + list(out_names)
    if partition_name is not None:
        all_in_names.append(partition_name)
    donate = tuple(range(n_params, n_params + n_outs))

    def _body(*args):
        operands = list(args)
        if partition_name is not None:
            operands.append(b2j.partition_id_tensor())
        outs = b2j._bass_exec_p.bind(
            *operands, out_avals=tuple(out_avals), in_names=tuple(all_in_names),
            out_names=tuple(out_names), lowering_input_output_aliases=(),
            sim_require_finite=True, sim_require_nnan=True, nc=nc)
        return tuple(outs)

    import jax as _jax
    devices = _jax.devices()[:n_cores]
    mesh = Mesh(np.asarray(devices), ("core",))
    in_specs = (PartitionSpec("core"),) * (n_params + n_outs)
    out_specs = (PartitionSpec("core"),) * n_outs
    sharded = _jax.jit(
        shard_map(_body, mesh=mesh, in_specs=in_specs, out_specs=out_specs,
                  check_rep=False),
        donate_argnums=donate, keep_unused=True)

    def run(in_maps):
        per_core = [[np.asarray(m[name]) for name in in_names] for m in in_maps]
        concat_in = [np.concatenate([per_core[c][i] for c in range(n_cores)],
                                    axis=0) for i in range(n_params)]
        concat_zeros = [np.zeros((n_cores * z.shape[0], *z.shape[1:]), z.dtype)
                        for z in zero_outs]
        out_arrs = sharded(*concat_in, *concat_zeros)
        return [
            {name: np.asarray(out_arrs[i]).reshape(n_cores, *out_avals[i].shape)[c]
             for i, name in enumerate(out_names)}
            for c in range(n_cores)
        ]

    return run


def _static_key(st):
    def freeze(v):
        if isinstance(v, dict):
            return tuple(sorted((k, freeze(x)) for k, x in v.items()))
        if isinstance(v, (list, tuple)):
            return tuple(freeze(x) for x in v)
        return v
    return freeze(st)


def kernel(**inputs) -> np.ndarray:
    static, in_maps = _host_prep(inputs)
    key = _static_key(static)
    if key not in _cache:
        nc = _build_program(static)
        _cache.clear()
        _cache[key] = _make_runner(nc, CORES)
        _cache[("nc", key)] = nc
    run = _cache[key]
    results = run(in_maps)
    out = np.concatenate([r["out"] for r in results], axis=0)
    return np.ascontiguousarray(out, np.float32)


# revision 10
# speedup vs baseline: 1.0218x; 1.0218x over previous
"""HAN heterogeneous-graph-attention kernel for 8 Trainium2 NeuronCores.

SPMD over 8 cores, destination-sharded edges:
  - Each core owns dst slice [c*S,(c+1)*S) of each node type; edges bucketed
    into 128-dst windows, each window padded to a core-uniform tile count.
  - Projections are node-sharded; a combined rhs [W | W@A_src.. | W@A_dst..]
    yields h plus all attention dot-products in one matmul.  h (bf16) and
    per-node a_src (fp32, bit-packed into bf16 slots) form gather-table rows;
    tables are AllGathered so gathers are local.
  - Per edge tile (128 edges): one indirect-DMA row gather; one-hot P from a
    DVE compare vs iota; a_dst selected via P^T matmul; exp(leaky(...)) with
    -30000 masking for pads; segment sums via P^T @ [e | e*h] matmuls
    accumulated per window in PSUM; epilogue divides by segment sum + relu.
  - Semantic attention via DMA-transpose reads + tanh/accum, AllReduce of
    per-core colsums, tiny on-device softmax; fusion + LayerNorm + relu;
    layer-2 projection fused per tile via PE transposes; final classifier.
"""
import sys
sys.path.insert(0, "/opt/trn_rl_repo")
sys.path.insert(0, "/root/.axon_site")

import numpy as np
import ml_dtypes

bf16 = ml_dtypes.bfloat16
P = 128
CORES = 8
HEADS = 8

_cache = {}


# ---------------------------------------------------------------------------
# host-side planning
# ---------------------------------------------------------------------------

def _amat(att_vec, C):
    H, D = att_vec.shape
    A = np.zeros((C, H), np.float32)
    for hd in range(H):
        A[hd * D:(hd + 1) * D, hd] = att_vec[hd]
    return A


def _plan_edge_type(src, dst, n_src, S_dst, n_cores):
    W = -(-S_dst // P)
    order = np.argsort(dst, kind="stable")
    ds = dst[order].astype(np.int64)
    ss = src[order].astype(np.int64)
    bounds = np.searchsorted(ds, np.arange(n_cores + 1) * S_dst)
    counts = np.zeros((n_cores, W), np.int64)
    per_core = []
    for c in range(n_cores):
        lo, hi = bounds[c], bounds[c + 1]
        d_loc = ds[lo:hi] - c * S_dst
        w = d_loc >> 7
        counts[c] = np.bincount(w, minlength=W)
        per_core.append((d_loc, ss[lo:hi], w))
    tpw = np.maximum(-(-counts.max(axis=0) // P), 1).astype(np.int64)
    T = int(tpw.sum())
    tw0 = np.concatenate([[0], np.cumsum(tpw)])
    cores = []
    for c in range(n_cores):
        d_loc, s_loc, w = per_core[c]
        src_a = np.zeros((P, T), np.int32)
        rel_a = np.zeros((P, T), np.float32)
        msk_a = np.full((P, T), -30000.0, np.float32)
        wstart = np.concatenate([[0], np.cumsum(counts[c])])
        rank = np.arange(len(w)) - wstart[w]
        col = (tw0[w] + (rank >> 7)).astype(np.int64)
        row = (rank & 127).astype(np.int64)
        src_a[row, col] = s_loc
        rel_a[row, col] = d_loc & 127
        msk_a[row, col] = 0.0
        cores.append(dict(src=src_a, rel=rel_a.astype(bf16), msk=msk_a))
    return dict(tpw=[int(x) for x in tpw], T=T, W=W, cores=cores)


def _host_prep(inputs):
    x_addr = np.asarray(inputs["x_addr"], np.float32)
    x_tx = np.asarray(inputs["x_tx"], np.float32)
    N_ADDR, F_IN = x_addr.shape
    N_TX = x_tx.shape[0]
    HID = np.asarray(inputs["W1_addr"]).shape[1]
    OUT = np.asarray(inputs["W2_addr"]).shape[1]
    NCLS = np.asarray(inputs["lin_W"]).shape[1]
    S = {"addr": N_ADDR // CORES, "tx": N_TX // CORES}
    N = {"addr": N_ADDR, "tx": N_TX}

    ETS = [("a2t", "addr", "tx"), ("t2a", "tx", "addr"),
           ("a2a", "addr", "addr"), ("t2t", "tx", "tx")]
    L1_ETS = [0, 1, 2, 3]
    L2_ETS = [1, 2]
    src_of = {nt: [i for i, (_, st_, _) in enumerate(ETS) if st_ == nt] for nt in S}
    dst_of = {nt: [i for i, (_, _, dt_) in enumerate(ETS) if dt_ == nt] for nt in S}
    src2_of = {nt: [i for i in L2_ETS if ETS[i][1] == nt] for nt in S}
    dst2_of = {nt: [i for i in L2_ETS if ETS[i][2] == nt] for nt in S}

    f32 = lambda k: np.asarray(inputs[k], np.float32)
    att1_src, att1_dst = f32("att1_src"), f32("att1_dst")
    att2_src, att2_dst = f32("att2_src"), f32("att2_dst")

    def build_rhs(Wm, bm, att_s, att_d, srcs, dsts, C):
        cols, bcols = [Wm], [bm]
        for i in srcs:
            A = _amat(att_s[i], C); cols.append(Wm @ A); bcols.append(bm @ A)
        for i in dsts:
            A = _amat(att_d[i], C); cols.append(Wm @ A); bcols.append(bm @ A)
        return (np.concatenate(cols, 1).astype(bf16),
                np.concatenate(bcols, 0).astype(np.float32))

    rhs1, bias1, rhs2, bias2 = {}, {}, {}, {}
    for nt, Wk, bk in [("addr", "W1_addr", "b1_addr"), ("tx", "W1_tx", "b1_tx")]:
        rhs1[nt], bias1[nt] = build_rhs(f32(Wk), f32(bk), att1_src, att1_dst,
                                        src_of[nt], dst_of[nt], HID)
    for nt, Wk, bk in [("addr", "W2_addr", "b2_addr"), ("tx", "W2_tx", "b2_tx")]:
        rhs2[nt], bias2[nt] = build_rhs(f32(Wk), f32(bk), att2_src, att2_dst,
                                        src2_of[nt], dst2_of[nt], OUT)

    plans = []
    for i, (name, st_, dt_) in enumerate(ETS):
        plans.append(_plan_edge_type(
            np.asarray(inputs[f"{name}_src"]), np.asarray(inputs[f"{name}_dst"]),
            N[st_], S[dt_], CORES))

    xT = {"addr": np.ascontiguousarray(x_addr.T).astype(bf16),
          "tx": np.ascontiguousarray(x_tx.T).astype(bf16)}

    pad512 = lambda s: -(-s // 512) * 512

    static = dict(
        N_ADDR=N_ADDR, N_TX=N_TX, F_IN=F_IN, HID=HID, OUT=OUT, NCLS=NCLS,
        S=S, N=N, ETS=ETS, L1_ETS=L1_ETS, L2_ETS=L2_ETS,
        src_of=src_of, dst_of=dst_of, src2_of=src2_of, dst2_of=dst2_of,
        tpw=[p["tpw"] for p in plans], T=[p["T"] for p in plans],
        W=[p["W"] for p in plans],
        bias1_nz={nt: bool(np.any(bias1[nt])) for nt in S},
        bias2_nz={nt: bool(np.any(bias2[nt])) for nt in S},
        kb1_nz=bool(np.any(f32("k1_b"))), kb2_nz=bool(np.any(f32("k2_b"))),
        g1_triv=bool(np.all(f32("ln1_g") == 1) and not np.any(f32("ln1_b"))),
        g2_triv=bool(np.all(f32("ln2_g") == 1) and not np.any(f32("ln2_b"))),
        linb_nz=bool(np.any(f32("lin_b"))),
        Spad={nt: pad512(S[nt]) for nt in S},
    )

    # q columns with 1/N(dst type of pair) folded in
    q1 = f32("q1"); q2 = f32("q2")
    nh1, nh2 = HID // P, OUT // P
    q1c = np.zeros((P, 2 * nh1 * 0 + len(L1_ETS) * nh1), np.float32)
    for pi, et in enumerate(L1_ETS):
        scale = 1.0 / N[ETS[et][2]]
        for h in range(nh1):
            q1c[:, pi * nh1 + h] = q1[h * P:(h + 1) * P] * scale
    q2c = np.zeros((P, len(L2_ETS) * nh2), np.float32)
    for pi, et in enumerate(L2_ETS):
        scale = 1.0 / N[ETS[et][2]]
        for h in range(nh2):
            q2c[:, pi * nh2 + h] = q2[h * P:(h + 1) * P] * scale

    kb1c = np.ascontiguousarray(f32("k1_b").reshape(nh1, P).T)  # [128, nh1]
    kb2c = np.ascontiguousarray(f32("k2_b").reshape(nh2, P).T)

    shared = {
        "rhs1_addr": rhs1["addr"], "rhs1_tx": rhs1["tx"],
        "rhs2_addr": rhs2["addr"], "rhs2_tx": rhs2["tx"],
        "bias1_addr": bias1["addr"][None, :].astype(bf16),
        "bias1_tx": bias1["tx"][None, :].astype(bf16),
        "bias2_addr": bias2["addr"][None, :].astype(bf16),
        "bias2_tx": bias2["tx"][None, :].astype(bf16),
        "kW1": f32("k1_W").astype(bf16), "kW2": f32("k2_W").astype(bf16),
        "kb1": kb1c, "kb2": kb2c,
        "q1cols": q1c, "q2cols": q2c,
        "ln1_g": f32("ln1_g")[None, :], "ln1_b": f32("ln1_b")[None, :],
        "ln2_g": f32("ln2_g")[None, :], "ln2_b": f32("ln2_b")[None, :],
        "lin_W": f32("lin_W").astype(bf16),
        "lin_b": f32("lin_b")[None, :].astype(bf16),
    }
    in_maps = []
    for c in range(CORES):
        m = dict(shared)
        m["xT_addr"] = np.ascontiguousarray(
            xT["addr"][:, c * S["addr"]:(c + 1) * S["addr"]])
        m["xT_tx"] = np.ascontiguousarray(
            xT["tx"][:, c * S["tx"]:(c + 1) * S["tx"]])
        for i, pl in enumerate(plans):
            m[f"esrc{i}"] = pl["cores"][c]["src"]
            m[f"erel{i}"] = pl["cores"][c]["rel"]
            m[f"emsk{i}"] = pl["cores"][c]["msk"]
        in_maps.append(m)
    return static, in_maps


# ---------------------------------------------------------------------------
# device program
# ---------------------------------------------------------------------------

def _build_program(st):
    import contextlib
    import concourse.bass as bass
    import concourse.mybir as mybir
    import concourse.tile as tile
    from concourse import bacc
    from concourse.masks import make_identity

    dt = mybir.dt
    AF = mybir.ActivationFunctionType
    OP = mybir.AluOpType
    X = mybir.AxisListType.X

    S, N = st["S"], st["N"]
    HID, OUT, F_IN, NCLS = st["HID"], st["OUT"], st["F_IN"], st["NCLS"]
    ETS = st["ETS"]
    nc = bacc.Bacc("TRN2", target_bir_lowering=False, debug=False,
                   num_devices=CORES)

    io = {}
    def ein(name, shape, dty):
        io[name] = nc.dram_tensor(name, shape, dty, kind="ExternalInput")

    n_as1 = {nt: len(st["src_of"][nt]) for nt in S}
    n_ad1 = {nt: len(st["dst_of"][nt]) for nt in S}
    n_as2 = {nt: len(st["src2_of"][nt]) for nt in S}
    n_ad2 = {nt: len(st["dst2_of"][nt]) for nt in S}
    C1cols = {nt: HID + 8 * (n_as1[nt] + n_ad1[nt]) for nt in S}
    C2cols = {nt: OUT + 8 * (n_as2[nt] + n_ad2[nt]) for nt in S}
    ROW1 = {nt: HID + 16 * n_as1[nt] for nt in S}
    ROW2 = {nt: OUT + 16 * n_as2[nt] for nt in S}
    nh1, nh2 = HID // P, OUT // P

    ein("xT_addr", [F_IN, S["addr"]], dt.bfloat16)
    ein("xT_tx", [F_IN, S["tx"]], dt.bfloat16)
    for nt in S:
        ein(f"rhs1_{nt}", [F_IN, C1cols[nt]], dt.bfloat16)
        ein(f"bias1_{nt}", [1, C1cols[nt]], dt.bfloat16)
        ein(f"rhs2_{nt}", [HID, C2cols[nt]], dt.bfloat16)
        ein(f"bias2_{nt}", [1, C2cols[nt]], dt.bfloat16)
    ein("kW1", [HID, HID], dt.bfloat16)
    ein("kW2", [OUT, OUT], dt.bfloat16)
    ein("kb1", [P, nh1], dt.float32)
    ein("kb2", [P, nh2], dt.float32)
    ein("q1cols", [P, len(st["L1_ETS"]) * nh1], dt.float32)
    ein("q2cols", [P, len(st["L2_ETS"]) * nh2], dt.float32)
    ein("ln1_g", [1, HID], dt.float32); ein("ln1_b", [1, HID], dt.float32)
    ein("ln2_g", [1, OUT], dt.float32); ein("ln2_b", [1, OUT], dt.float32)
    ein("lin_W", [OUT, NCLS], dt.bfloat16)
    ein("lin_b", [1, NCLS], dt.bfloat16)
    for i in range(4):
        ein(f"esrc{i}", [P, st["T"][i]], dt.int32)
        ein(f"erel{i}", [P, st["T"][i]], dt.bfloat16)
        ein(f"emsk{i}", [P, st["T"][i]], dt.float32)
    out_t = nc.dram_tensor("out", [S["addr"], NCLS], dt.float32,
                           kind="ExternalOutput")
    RG = [list(range(CORES))]

    with tile.TileContext(nc) as tc, contextlib.ExitStack() as ctx:
        dram = ctx.enter_context(tc.tile_pool(name="dram", bufs=1, space="DRAM"))
        const = ctx.enter_context(tc.tile_pool(name="const", bufs=1))

        # ---- persistent DRAM ----
        Wn = {nt: -(-S[nt] // P) for nt in S}
        tab1_loc = {nt: dram.tile([S[nt], ROW1[nt]], dt.bfloat16,
                                  name=f"tab1loc_{nt}") for nt in S}
        tab1 = {nt: dram.tile([N[nt], ROW1[nt]], dt.bfloat16,
                              addr_space="Shared", name=f"tab1_{nt}") for nt in S}
        tab2_loc = {nt: dram.tile([S[nt], ROW2[nt]], dt.bfloat16,
                                  name=f"tab2loc_{nt}") for nt in S}
        tab2 = {nt: dram.tile([N[nt], ROW2[nt]], dt.bfloat16,
                              addr_space="Shared", name=f"tab2_{nt}") for nt in S}
        ad1 = {nt: dram.tile([Wn[nt] * P, 16], dt.float32, name=f"ad1_{nt}")
               for nt in S}
        ad2 = {"addr": dram.tile([Wn["addr"] * P, 16], dt.float32, name="ad2_addr")}
        o1 = {i: dram.tile([st["Spad"][ETS[i][2]], HID], dt.bfloat16,
                           name=f"o1_{i}") for i in st["L1_ETS"]}
        o2 = {i: dram.tile([st["Spad"]["addr"], OUT], dt.bfloat16,
                           name=f"o2_{i}") for i in st["L2_ETS"]}
        npair1 = len(st["L1_ETS"]) * nh1
        npair2 = len(st["L2_ETS"]) * nh2
        sc1_in = dram.tile([P, npair1], dt.float32, name="sc1_in")
        sc1_out = dram.tile([P, npair1], dt.float32, addr_space="Shared",
                            name="sc1_out")
        sc2_in = dram.tile([P, npair2], dt.float32, name="sc2_in")
        sc2_out = dram.tile([P, npair2], dt.float32, addr_space="Shared",
                            name="sc2_out")

        # ---- constants ----
        iota_row = const.tile([P, P], dt.bfloat16)
        nc.gpsimd.iota(iota_row[:], pattern=[[1, P]], base=0,
                       channel_multiplier=0, allow_small_or_imprecise_dtypes=True)
        ident = const.tile([P, P], dt.bfloat16)
        make_identity(nc, ident[:])
        ones_row = const.tile([1, P], dt.bfloat16)
        nc.vector.memset(ones_row[:], 1.0)
        ones_row_f = const.tile([1, P], dt.float32)
        nc.vector.memset(ones_row_f[:], 1.0)
        ones_col_f = const.tile([P, 1], dt.float32)
        nc.vector.memset(ones_col_f[:], 1.0)
        eps_ln = const.tile([P, 1], dt.float32)
        nc.vector.memset(eps_ln[:], 1e-5)
        zrow = const.tile([P, 640], dt.bfloat16)
        nc.vector.memset(zrow[:], 0.0)
        zrow_f = const.tile([P, 16], dt.float32)
        nc.vector.memset(zrow_f[:], 0.0)

        for nt in S:
            padn = Wn[nt] * P - S[nt]
            if padn:
                nc.sync.dma_start(ad1[nt][S[nt]:, :], zrow_f[:padn, :])
                if nt in ad2:
                    nc.sync.dma_start(ad2[nt][S[nt]:, :], zrow_f[:padn, :])
        for i, o in o1.items():
            Sr = S[ETS[i][2]]
            padn = st["Spad"][ETS[i][2]] - Sr
            for r0 in range(0, padn, P):
                rr = min(P, padn - r0)
                nc.sync.dma_start(o[Sr + r0:Sr + r0 + rr, :], zrow[:rr, :HID])
        for i, o in o2.items():
            Sr = S["addr"]
            padn = st["Spad"]["addr"] - Sr
            for r0 in range(0, padn, P):
                rr = min(P, padn - r0)
                nc.sync.dma_start(o[Sr + r0:Sr + r0 + rr, :], zrow[:rr, :OUT])

        # ---- weights in SBUF ----
        sb = {}
        def load_blocks(key, R, C):
            ts = []
            for k in range(-(-R // P)):
                r = min(P, R - k * P)
                t = const.tile([r, C], dt.bfloat16, name=f"sb_{key}_{k}")
                nc.sync.dma_start(t[:], io[key][k * P:k * P + r, :])
                ts.append(t)
            return ts
        for nt in S:
            sb[f"rhs1_{nt}"] = load_blocks(f"rhs1_{nt}", F_IN, C1cols[nt])
            sb[f"rhs2_{nt}"] = load_blocks(f"rhs2_{nt}", HID, C2cols[nt])
            for b in (f"bias1_{nt}", f"bias2_{nt}"):
                C = C1cols[nt] if b.startswith("bias1") else C2cols[nt]
                t = const.tile([1, C], dt.bfloat16, name=f"sb_{b}")
                nc.sync.dma_start(t[:], io[b][:])
                sb[b] = t
        sb["kW1"] = load_blocks("kW1", HID, HID)
        sb["kW2"] = load_blocks("kW2", OUT, OUT)
        sb["lin_W"] = load_blocks("lin_W", OUT, NCLS)
        t = const.tile([1, NCLS], dt.bfloat16, name="sb_linb")
        nc.sync.dma_start(t[:], io["lin_b"][:])
        sb["lin_b"] = t
        for k, sh in [("kb1", [P, nh1]), ("kb2", [P, nh2]),
                      ("q1cols", [P, npair1]), ("q2cols", [P, npair2])]:
            t = const.tile(sh, dt.float32, name=f"sb_{k}")
            nc.sync.dma_start(t[:], io[k][:])
            sb[k] = t

        ln_bc = {}
        for L, C, triv in [(1, HID, st["g1_triv"]), (2, OUT, st["g2_triv"])]:
            if triv:
                continue
            with tc.tile_pool(name=f"lnp{L}", bufs=2, space="PSUM") as lnp:
                for suffix in ("g", "b"):
                    k = f"ln{L}_{suffix}"
                    row = const.tile([1, C], dt.float32, name=f"row_{k}")
                    nc.sync.dma_start(row[:], io[k][:])
                    ps = lnp.tile([P, C], dt.float32, space="PSUM", name="lnps")
                    nc.tensor.matmul(ps[:], ones_row_f[:], row[:],
                                     start=True, stop=True)
                    t = const.tile([P, C], dt.float32, name=f"bc_{k}")
                    nc.vector.tensor_copy(t[:], ps[:])
                    ln_bc[k] = t

        meta = {}
        for i in range(4):
            for pre, dty in [("esrc", dt.int32), ("erel", dt.bfloat16),
                             ("emsk", dt.float32)]:
                t = const.tile([P, st["T"][i]], dty, name=f"{pre}{i}")
                nc.sync.dma_start(t[:], io[f"{pre}{i}"][:])
                meta[f"{pre}{i}"] = t

        # =========================================================
        def proj_pass(L, nt, get_lhsT, Ccols, Crow, Cdim, tab_loc_t, ad_t,
                      n_as, n_ad, bias_nz, extra_cb=None):
            n_tiles = -(-S[nt] // P)
            with (
                tc.tile_pool(name=f"pp{L}{nt}", bufs=3) as pp,
                tc.tile_pool(name=f"pps{L}{nt}", bufs=4, space="PSUM") as pps,
            ):
                SLAB = 8
                out_slab = ad_slab = None
                for i in range(n_tiles):
                    rows = min(P, S[nt] - i * P)
                    if i % SLAB == 0:
                        out_slab = pp.tile([P, SLAB, Crow], dt.bfloat16,
                                           name="oslab")
                        ad_slab = (pp.tile([P, SLAB, 16], dt.float32,
                                           name="adslab") if n_ad else None)
                    j = i % SLAB
                    ps = pps.tile([P, Ccols], dt.float32, space="PSUM",
                                  name="projps")
                    blocks = get_lhsT(i, pp, pps)
                    nb = len(blocks) + (1 if bias_nz else 0)
                    for bi, lhsT in enumerate(blocks):
                        nc.tensor.matmul(ps[:rows, :], lhsT,
                                         sb[f"rhs{L}_{nt}"][bi][:],
                                         start=(bi == 0), stop=(bi == nb - 1))
                    if bias_nz:
                        nc.tensor.matmul(ps[:rows, :], ones_row[:, :rows],
                                         sb[f"bias{L}_{nt}"][:],
                                         start=False, stop=True)
                    nc.any.tensor_copy(out_slab[:rows, j, 0:Cdim],
                                       ps[:rows, 0:Cdim])
                    if n_as:
                        nc.vector.tensor_copy(
                            out_slab[:rows, j, Cdim:Cdim + 16 * n_as]
                            .bitcast(dt.float32),
                            ps[:rows, Cdim:Cdim + 8 * n_as])
                    if n_ad:
                        nc.vector.tensor_copy(
                            ad_slab[:rows, j, 0:8 * n_ad],
                            ps[:rows, Cdim + 8 * n_as:Cdim + 8 * (n_as + n_ad)])
                        if n_ad == 1:
                            nc.vector.memset(ad_slab[:rows, j, 8:16], 0.0)
                    if j == SLAB - 1 or i == n_tiles - 1:
                        i0 = (i // SLAB) * SLAB
                        k = i - i0 + 1
                        kf = k - (0 if rows == P else 1)
                        if kf:
                            nc.sync.dma_start(
                                tab_loc_t[i0 * P:i0 * P + kf * P, :]
                                .rearrange("(k p) r -> p k r", p=P),
                                out_slab[:, 0:kf, :])
                            if n_ad:
                                nc.sync.dma_start(
                                    ad_t[i0 * P:i0 * P + kf * P, :]
                                    .rearrange("(k p) r -> p k r", p=P),
                                    ad_slab[:, 0:kf, :])
                        if rows < P:
                            nc.sync.dma_start(tab_loc_t[i * P:i * P + rows, :],
                                              out_slab[:rows, k - 1, :])
                            if n_ad:
                                nc.sync.dma_start(ad_t[i * P:i * P + rows, :],
                                                  ad_slab[:rows, k - 1, :])

        # ---- phase 1: layer-1 projections ----
        with tc.tile_pool(name="xts", bufs=3) as xts:
            for nt in S:
                state = {}

                def get_lhsT1(i, pp, pps, nt=nt, state=state):
                    GS = 8
                    g = i // GS
                    if state.get("g") != g:
                        cols = min(GS * P, S[nt] - g * GS * P)
                        tl = xts.tile([F_IN, GS * P], dt.bfloat16,
                                      name=f"xt_{nt}")
                        nc.sync.dma_start(
                            tl[:, :cols],
                            io[f"xT_{nt}"][:, g * GS * P:g * GS * P + cols])
                        state["g"] = g
                        state["t"] = tl
                    off = (i % GS) * P
                    cols = min(P, S[nt] - i * P)
                    return [state["t"][:, off:off + cols]]

                proj_pass(1, nt, get_lhsT1, C1cols[nt], ROW1[nt], HID,
                          tab1_loc[nt], ad1[nt], n_as1[nt], n_ad1[nt],
                          st["bias1_nz"][nt])
        # =========================================================
        def edge_pass(L, ets, tabs, ROWt, ads, os_, Cdim, asrc_idx, adcol_idx):
            D = Cdim // HEADS
            BMAX = 12
            for et in ets:
                _, stp, dtp = ETS[et]
                tpw = st["tpw"][et]
                Srows = S[dtp]
                tw0 = np.concatenate([[0], np.cumsum(tpw)]).astype(int)
                batches, cur, curB = [], [], 0
                for w in range(st["W"][et]):
                    if cur and (curB + tpw[w] > BMAX or len(cur) >= 4):
                        batches.append(cur); cur, curB = [], 0
                    cur.append(w); curB += tpw[w]
                if cur:
                    batches.append(cur)
                aoff = asrc_idx[et] * 16
                acol = adcol_idx[et] * 8
                with (
                    tc.tile_pool(name=f"eg{L}{et}", bufs=3) as eg,
                    tc.tile_pool(name=f"eps{L}{et}", bufs=4, space="PSUM") as eps1,
                    tc.tile_pool(name=f"ep2{L}{et}", bufs=2, space="PSUM") as eps2,
                ):
                    for wins in batches:
                        t0, t1 = int(tw0[wins[0]]), int(tw0[wins[-1] + 1])
                        B = t1 - t0
                        w0, nw = wins[0], len(wins)
                        G = eg.tile([P, B, ROWt[stp]], dt.bfloat16, name="G")
                        for b in range(B):
                            nc.gpsimd.indirect_dma_start(
                                out=G[:, b, :], out_offset=None,
                                in_=tabs[stp][:],
                                in_offset=bass.IndirectOffsetOnAxis(
                                    ap=meta[f"esrc{et}"][:, t0 + b:t0 + b + 1],
                                    axis=0))
                        adw = eg.tile([P, nw, 16], dt.float32, name="adw")
                        nc.sync.dma_start(
                            adw[:], ads[dtp][w0 * P:(w0 + nw) * P, :]
                            .rearrange("(k p) r -> p k r", p=P))
                        adw_bf = eg.tile([P, nw, 16], dt.bfloat16, name="adwbf")
                        nc.vector.tensor_copy(adw_bf[:], adw[:])
                        Pm = eg.tile([P, B, P], dt.bfloat16, name="Pm")
                        nc.vector.tensor_tensor(
                            out=Pm[:],
                            in0=meta[f"erel{et}"][:, t0:t1, None]
                            .to_broadcast([P, B, P]),
                            in1=iota_row[:, None, :].to_broadcast([P, B, P]),
                            op=OP.is_equal)
                        ad_ps = eps2.tile([P, B, 8], dt.float32, space="PSUM",
                                          name="adps")
                        for bw, w in enumerate(wins):
                            for b in range(tw0[w] - t0, tw0[w + 1] - t0):
                                pt_ps = eps2.tile([P, P], dt.bfloat16,
                                                  space="PSUM", name="ptps")
                                nc.tensor.transpose(pt_ps[:], Pm[:, b, :],
                                                    ident[:])
                                pt_sb = eg.tile([P, P], dt.bfloat16, name="ptsb")
                                nc.vector.tensor_copy(pt_sb[:], pt_ps[:])
                                nc.tensor.matmul(
                                    ad_ps[:, b, :], pt_sb[:],
                                    adw_bf[:, bw, acol:acol + 8],
                                    start=True, stop=True,
                                    skip_group_check=True)
                        alpha = eg.tile([P, B, 8], dt.float32, name="alpha")
                        nc.vector.tensor_tensor(
                            out=alpha[:],
                            in0=G[:, :, Cdim + aoff:Cdim + aoff + 16]
                            .bitcast(dt.float32),
                            in1=ad_ps[:], op=OP.add)
                        nc.vector.tensor_tensor(
                            out=alpha[:], in0=alpha[:],
                            in1=meta[f"emsk{et}"][:, t0:t1, None]
                            .to_broadcast([P, B, 8]), op=OP.add)
                        lr = eg.tile([P, B, 8], dt.float32, name="lr")
                        nc.vector.scalar_tensor_tensor(
                            lr[:], alpha[:], 0.2, alpha[:],
                            op0=OP.mult, op1=OP.max)
                        msg = eg.tile([P, B, 8 + Cdim], dt.bfloat16, name="msg")
                        nc.scalar.activation(msg[:, :, 0:8], lr[:], AF.Exp)
                        nc.vector.tensor_tensor(
                            out=msg[:, :, 8:8 + Cdim]
                            .rearrange("p b (h d) -> p b h d", h=HEADS),
                            in0=G[:, :, 0:Cdim]
                            .rearrange("p b (h d) -> p b h d", h=HEADS),
                            in1=msg[:, :, 0:8][:, :, :, None]
                            .to_broadcast([P, B, HEADS, D]),
                            op=OP.mult)
                        o_slab = eg.tile([P, nw, Cdim], dt.bfloat16,
                                         name="oslabE")
                        for bw, w in enumerate(wins):
                            ps = eps1.tile([P, 8 + Cdim], dt.float32,
                                           space="PSUM", name="aggps")
                            bs = list(range(tw0[w] - t0, tw0[w + 1] - t0))
                            for k, b in enumerate(bs):
                                nc.tensor.matmul(ps[:], Pm[:, b, :],
                                                 msg[:, b, :],
                                                 start=(k == 0),
                                                 stop=(k == len(bs) - 1))
                            srec = eg.tile([P, 8], dt.float32, name="srec")
                            nc.vector.tensor_scalar(srec[:], ps[:, 0:8],
                                                    1e-16, None, op0=OP.add)
                            nc.vector.reciprocal(srec[:], srec[:])
                            nc.vector.tensor_tensor(
                                out=o_slab[:, bw, :]
                                .rearrange("p (h d) -> p h d", h=HEADS),
                                in0=ps[:, 8:8 + Cdim]
                                .rearrange("p (h d) -> p h d", h=HEADS),
                                in1=srec[:, :, None]
                                .to_broadcast([P, HEADS, D]),
                                op=OP.mult)
                        nc.vector.tensor_scalar(o_slab[:], o_slab[:], 0.0,
                                                None, op0=OP.max)
                        lastw = wins[-1]
                        full = nw - (1 if (lastw + 1) * P > Srows else 0)
                        if full:
                            nc.sync.dma_start(
                                os_[et][w0 * P:(w0 + full) * P, :]
                                .rearrange("(k p) r -> p k r", p=P),
                                o_slab[:, 0:full, :])
                        if full < nw:
                            rr = Srows - lastw * P
                            nc.sync.dma_start(os_[et][lastw * P:Srows, :],
                                              o_slab[:rr, nw - 1, :])

        # =========================================================
        def score_et(L, et, pi, acc, os_, Cdim, kWk, kbk, kb_nz):
            nh = Cdim // P
            with (
                tc.tile_pool(name=f"sp{L}{et}", bufs=3) as sp,
                tc.tile_pool(name=f"sps{L}{et}", bufs=4, space="PSUM") as sps,
            ):
                if True:
                    dtp = ETS[et][2]
                    Srp = st["Spad"][dtp]
                    for n0 in range(0, Srp, 512):
                        oTs = []
                        for k in range(nh):
                            oT = sp.tile([P, 512], dt.bfloat16, name="oT")
                            nc.sync.dma_start_transpose(
                                out=oT[:],
                                in_=os_[et][n0:n0 + 512, k * P:(k + 1) * P])
                            oTs.append(oT)
                        for h in range(nh):
                            zps = sps.tile([P, 512], dt.float32, space="PSUM",
                                           name="zps")
                            for k in range(nh):
                                nc.tensor.matmul(
                                    zps[:], sb[kWk][k][:, h * P:(h + 1) * P],
                                    oTs[k][:], start=(k == 0),
                                    stop=(k == nh - 1))
                            th = sp.tile([P, 512], dt.float32, name="th")
                            chs = sp.tile([P, 1], dt.float32, name="chs")
                            if kb_nz:
                                nc.scalar.activation(th[:], zps[:], AF.Tanh,
                                                     bias=sb[kbk][:, h:h + 1],
                                                     accum_out=chs[:])
                            else:
                                nc.scalar.activation(th[:], zps[:], AF.Tanh,
                                                     accum_out=chs[:])
                            nc.vector.tensor_tensor(
                                acc[:, pi * nh + h:pi * nh + h + 1],
                                acc[:, pi * nh + h:pi * nh + h + 1],
                                chs[:], op=OP.add)
                    if kb_nz:
                        npad = Srp - S[dtp]
                        if npad:
                            for h in range(nh):
                                kbt = sp.tile([P, 1], dt.float32, name="kbt")
                                nc.scalar.activation(kbt[:],
                                                     sb[kbk][:, h:h + 1],
                                                     AF.Tanh)
                                nc.vector.scalar_tensor_tensor(
                                    acc[:, pi * nh + h:pi * nh + h + 1],
                                    kbt[:], -float(npad),
                                    acc[:, pi * nh + h:pi * nh + h + 1],
                                    op0=OP.mult, op1=OP.add)
        def score_finish(L, ets, Cdim, acc, qk, sc_in, sc_out):
            nh = Cdim // P
            with tc.tile_pool(name=f"sf{L}", bufs=1) as sp:
                qprod = sp.tile([P, len(ets) * nh], dt.float32, name="qprod")
                nc.vector.tensor_tensor(qprod[:], acc[:], sb[qk][:], op=OP.mult)
                nc.sync.dma_start(sc_in[:], qprod[:])
            nc.gpsimd.collective_compute(
                "AllReduce", OP.add, replica_groups=RG,
                ins=[sc_in[:]], outs=[sc_out[:]])

        def softmax_weights(L, ets, groups, sc_out, Cdim):
            """groups: dict nt -> (pi_a, pi_b). Returns w_bc [128, npair] f32."""
            nh = Cdim // P
            npair = len(ets) * nh
            with (
                tc.tile_pool(name=f"sw{L}", bufs=1) as sw,
                tc.tile_pool(name=f"swp{L}", bufs=2, space="PSUM") as swp,
            ):
                scs = sw.tile([P, npair], dt.float32, name="scs")
                nc.sync.dma_start(scs[:], sc_out[:])
                rps = swp.tile([1, npair], dt.float32, space="PSUM", name="rps")
                nc.tensor.matmul(rps[:], ones_col_f[:], scs[:],
                                 start=True, stop=True)
                srow = sw.tile([1, len(ets)], dt.float32, name="srow")
                if nh > 1:
                    nc.vector.tensor_reduce(
                        srow[:], rps[:].rearrange("o (e h) -> o e h", h=nh),
                        axis=X, op=OP.add)
                else:
                    nc.vector.tensor_copy(srow[:], rps[:])
                wrow = sw.tile([1, len(ets)], dt.float32, name="wrow")
                for nt, (pa, pb) in groups.items():
                    m = sw.tile([1, 1], dt.float32, name="m")
                    nc.vector.tensor_tensor(m[:], srow[:, pa:pa + 1],
                                            srow[:, pb:pb + 1], op=OP.max)
                    ea = sw.tile([1, 1], dt.float32, name="ea")
                    eb = sw.tile([1, 1], dt.float32, name="eb")
                    da = sw.tile([1, 1], dt.float32, name="da")
                    db = sw.tile([1, 1], dt.float32, name="db")
                    nc.vector.tensor_tensor(da[:], srow[:, pa:pa + 1], m[:],
                                            op=OP.subtract)
                    nc.vector.tensor_tensor(db[:], srow[:, pb:pb + 1], m[:],
                                            op=OP.subtract)
                    nc.scalar.activation(ea[:], da[:], AF.Exp)
                    nc.scalar.activation(eb[:], db[:], AF.Exp)
                    ssum = sw.tile([1, 1], dt.float32, name="ssum")
                    nc.vector.tensor_tensor(ssum[:], ea[:], eb[:], op=OP.add)
                    nc.vector.reciprocal(ssum[:], ssum[:])
                    nc.vector.tensor_tensor(wrow[:, pa:pa + 1], ea[:], ssum[:],
                                            op=OP.mult)
                    nc.vector.tensor_tensor(wrow[:, pb:pb + 1], eb[:], ssum[:],
                                            op=OP.mult)
                wps = swp.tile([P, len(ets)], dt.float32, space="PSUM",
                               name="wps")
                nc.tensor.matmul(wps[:], ones_row_f[:], wrow[:],
                                 start=True, stop=True)
                w_bc = const.tile([P, len(ets)], dt.float32, name=f"wbc{L}")
                nc.vector.tensor_copy(w_bc[:], wps[:])
            return w_bc

        def fuse_ln(pp, pps, i, nt, oa, ob, w_bc, pa, pb, Cdim, g_triv, L):
            """Load o tiles, fuse, LN, relu -> bf16 tile [rows, Cdim]."""
            rows = min(P, S[nt] - i * P)
            ta = pp.tile([P, Cdim], dt.bfloat16, name="fl_oa")
            tb = pp.tile([P, Cdim], dt.bfloat16, name="fl_ob")
            nc.sync.dma_start(ta[:rows], oa[i * P:i * P + rows, :])
            nc.sync.dma_start(tb[:rows], ob[i * P:i * P + rows, :])
            fused = pp.tile([P, Cdim], dt.float32, name="fl_fused")
            nc.vector.tensor_scalar(fused[:rows], ta[:rows],
                                    w_bc[:rows, pa:pa + 1], None, op0=OP.mult)
            nc.vector.scalar_tensor_tensor(fused[:rows], tb[:rows],
                                           w_bc[:rows, pb:pb + 1], fused[:rows],
                                           op0=OP.mult, op1=OP.add)
            srow = pp.tile([P, 1], dt.float32, name="fl_srow")
            cpscr = pp.tile([P, Cdim], dt.float32, name="fl_cpscr")
            nc.scalar.activation(cpscr[:rows], fused[:rows], AF.Copy,
                                 accum_out=srow[:rows])
            mu = pp.tile([P, 1], dt.float32, name="fl_mu")
            nc.vector.tensor_scalar(mu[:rows], srow[:rows], 1.0 / Cdim, None,
                                    op0=OP.mult)
            sq = pp.tile([P, Cdim], dt.float32, name="fl_sq")
            ssq = pp.tile([P, 1], dt.float32, name="fl_ssq")
            nc.scalar.activation(sq[:rows], fused[:rows], AF.Square,
                                 accum_out=ssq[:rows])
            musq = pp.tile([P, 1], dt.float32, name="fl_musq")
            nc.vector.tensor_tensor(musq[:rows], mu[:rows], mu[:rows],
                                    op=OP.mult)
            var = pp.tile([P, 1], dt.float32, name="fl_var")
            nc.vector.scalar_tensor_tensor(var[:rows], ssq[:rows], 1.0 / Cdim,
                                           musq[:rows], op0=OP.mult,
                                           op1=OP.subtract)
            nc.vector.tensor_scalar(var[:rows], var[:rows], 0.0, None,
                                    op0=OP.max)
            stdt = pp.tile([P, 1], dt.float32, name="fl_std")
            nc.scalar.activation(stdt[:rows], var[:rows], AF.Sqrt,
                                 bias=eps_ln[:rows])
            rstd = pp.tile([P, 1], dt.float32, name="fl_rstd")
            nc.vector.reciprocal(rstd[:rows], stdt[:rows])
            xn = pp.tile([P, Cdim], dt.float32, name="fl_xn")
            nc.vector.tensor_scalar(xn[:rows], fused[:rows], mu[:rows],
                                    rstd[:rows], op0=OP.subtract, op1=OP.mult)
            if not g_triv:
                nc.vector.tensor_tensor(xn[:rows], xn[:rows],
                                        ln_bc[f"ln{L}_g"][:rows], op=OP.mult)
                nc.vector.tensor_tensor(xn[:rows], xn[:rows],
                                        ln_bc[f"ln{L}_b"][:rows], op=OP.add)
            h_bf = pp.tile([P, Cdim], dt.bfloat16, name="fl_hbf")
            nc.scalar.activation(h_bf[:rows], xn[:rows], AF.Relu)
            return h_bf

        # ---- layer 1: per-type AllGather, edges ordered by src table,
        #      score accumulation interleaved per edge type ----
        asrc_idx1 = {et: st["src_of"][ETS[et][1]].index(et)
                     for et in st["L1_ETS"]}
        adcol_idx1 = {et: st["dst_of"][ETS[et][2]].index(et)
                      for et in st["L1_ETS"]}
        acc1 = const.tile([P, len(st["L1_ETS"]) * (HID // P)], dt.float32,
                          name="scacc1")
        nc.vector.memset(acc1[:], 0.0)
        ets_order1 = ([et for et in st["L1_ETS"] if ETS[et][1] == "addr"] +
                      [et for et in st["L1_ETS"] if ETS[et][1] == "tx"])
        for nt in ("addr", "tx"):
            nc.gpsimd.collective_compute(
                "AllGather", OP.bypass, replica_groups=RG,
                ins=[tab1_loc[nt][:]], outs=[tab1[nt][0:N[nt], :]])
        for et in ets_order1:
            edge_pass(1, [et], tab1, ROW1, ad1, o1, HID,
                      asrc_idx1, adcol_idx1)
            score_et(1, et, st["L1_ETS"].index(et), acc1, o1, HID,
                     "kW1", "kb1", st["kb1_nz"])
        score_finish(1, st["L1_ETS"], HID, acc1, "q1cols", sc1_in, sc1_out)
        groups1 = {}
        for nt in S:
            pair = st["dst_of"][nt]
            groups1[nt] = (st["L1_ETS"].index(pair[0]),
                           st["L1_ETS"].index(pair[1]))
        w1_bc = softmax_weights(1, st["L1_ETS"], groups1, sc1_out, HID)

        # ---- layer 1 fusion + LN + layer-2 projection ----
        for nt in S:
            pa, pb = groups1[nt]
            eta, etb = st["dst_of"][nt]

            def get_lhsT2(i, pp, pps, nt=nt, pa=pa, pb=pb, eta=eta, etb=etb):
                h_bf = fuse_ln(pp, pps, i, nt, o1[eta], o1[etb], w1_bc,
                               pa, pb, HID, st["g1_triv"], 1)
                rows = min(P, S[nt] - i * P)
                blocks = []
                for k in range(HID // P):
                    tp_ps = pps.tile([P, P], dt.bfloat16, space="PSUM",
                                     name="tpps", bufs=2)
                    nc.tensor.transpose(tp_ps[:, :rows],
                                        h_bf[:rows, k * P:(k + 1) * P],
                                        ident[:rows, :rows])
                    tsb = pp.tile([P, P], dt.bfloat16, name=f"h1T_{k}")
                    nc.vector.tensor_copy(tsb[:, :rows], tp_ps[:, :rows])
                    blocks.append(tsb[:, :rows])
                return blocks

            proj_pass(2, nt, get_lhsT2, C2cols[nt], ROW2[nt], OUT,
                      tab2_loc[nt], ad2.get(nt), n_as2[nt], n_ad2[nt],
                      st["bias2_nz"][nt])
        # ---- layer 2: AG per type (tx first, its edges run first), edges
        #      interleaved with score accumulation ----
        asrc_idx2 = {et: st["src2_of"][ETS[et][1]].index(et)
                     for et in st["L2_ETS"]}
        adcol_idx2 = {et: st["dst2_of"][ETS[et][2]].index(et)
                      for et in st["L2_ETS"]}
        acc2 = const.tile([P, len(st["L2_ETS"]) * (OUT // P)], dt.float32,
                          name="scacc2")
        nc.vector.memset(acc2[:], 0.0)
        ets_order2 = ([et for et in st["L2_ETS"] if ETS[et][1] == "tx"] +
                      [et for et in st["L2_ETS"] if ETS[et][1] == "addr"])
        for nt in ("tx", "addr"):
            nc.gpsimd.collective_compute(
                "AllGather", OP.bypass, replica_groups=RG,
                ins=[tab2_loc[nt][:]], outs=[tab2[nt][0:N[nt], :]])
        for et in ets_order2:
            edge_pass(2, [et], tab2, ROW2, ad2, o2, OUT,
                      asrc_idx2, adcol_idx2)
            score_et(2, et, st["L2_ETS"].index(et), acc2, o2, OUT,
                     "kW2", "kb2", st["kb2_nz"])
        score_finish(2, st["L2_ETS"], OUT, acc2, "q2cols", sc2_in, sc2_out)
        pair2 = st["dst2_of"]["addr"]
        groups2 = {"addr": (st["L2_ETS"].index(pair2[0]),
                            st["L2_ETS"].index(pair2[1]))}
        w2_bc = softmax_weights(2, st["L2_ETS"], groups2, sc2_out, OUT)

        # ---- layer 2 fusion + classifier ----
        pa2, pb2 = groups2["addr"]
        eta2, etb2 = pair2
        n_tiles = -(-S["addr"] // P)
        with (
            tc.tile_pool(name="cls", bufs=3) as cls,
            tc.tile_pool(name="clsp", bufs=4, space="PSUM") as clsp,
        ):
            SLAB = 8
            out_slab = None
            for i in range(n_tiles):
                rows = min(P, S["addr"] - i * P)
                if i % SLAB == 0:
                    out_slab = cls.tile([P, SLAB, NCLS], dt.float32,
                                        name="cl_oslab")
                j = i % SLAB
                h_bf = fuse_ln(cls, clsp, i, "addr", o2[eta2], o2[etb2],
                               w2_bc, pa2, pb2, OUT, st["g2_triv"], 2)
                ps = clsp.tile([P, NCLS], dt.float32, space="PSUM",
                               name="cl_ps")
                nbk = OUT // P
                nb = nbk + (1 if st["linb_nz"] else 0)
                for k in range(nbk):
                    tp_ps = clsp.tile([P, P], dt.bfloat16, space="PSUM",
                                      name="cl_tp", bufs=2)
                    nc.tensor.transpose(tp_ps[:, :rows],
                                        h_bf[:rows, k * P:(k + 1) * P],
                                        ident[:rows, :rows])
                    tsb = cls.tile([P, P], dt.bfloat16, name="cl_h2T")
                    nc.vector.tensor_copy(tsb[:, :rows], tp_ps[:, :rows])
                    nc.tensor.matmul(ps[:rows], tsb[:, :rows],
                                     sb["lin_W"][k][:],
                                     start=(k == 0), stop=(k == nb - 1))
                if st["linb_nz"]:
                    nc.tensor.matmul(ps[:rows], ones_row[:, :rows],
                                     sb["lin_b"][:], start=False, stop=True)
                nc.vector.tensor_copy(out_slab[:rows, j, :], ps[:rows])
                if j == SLAB - 1 or i == n_tiles - 1:
                    i0 = (i // SLAB) * SLAB
                    k = i - i0 + 1
                    kf = k - (0 if rows == P else 1)
                    if kf:
                        nc.sync.dma_start(
                            out_t[i0 * P:i0 * P + kf * P, :]
                            .rearrange("(k p) r -> p k r", p=P),
                            out_slab[:, 0:kf, :])
                    if rows < P:
                        nc.sync.dma_start(out_t[i * P:i * P + rows, :],
                                          out_slab[:rows, k - 1, :])

    nc.compile()
    return nc


# ---------------------------------------------------------------------------
# runner (cached compile via persistent jitted callable)
# ---------------------------------------------------------------------------

def _make_runner(nc, n_cores):
    import jax
    from jax.sharding import Mesh, PartitionSpec
    from jax.experimental.shard_map import shard_map
    import concourse.mybir as mybir
    import concourse.bass2jax as b2j

    b2j.install_neuronx_cc_hook()
    partition_name = nc.partition_id_tensor.name if nc.partition_id_tensor else None
    in_names, out_names, out_avals, zero_outs = [], [], [], []
    for alloc in nc.m.functions[0].allocations:
        if not isinstance(alloc, mybir.MemoryLocationSet):
            continue
        name = alloc.memorylocations[0].name
        if alloc.kind == "ExternalInput":
            if name != partition_name:
                in_names.append(name)
        elif alloc.kind == "ExternalOutput":
            shape = tuple(alloc.tensor_shape)
            dtype = mybir.dt.np(alloc.dtype)
            out_names.append(name)
            out_avals.append(jax.core.ShapedArray(shape, dtype))
            zero_outs.append(np.zeros(shape, dtype))
    n_params = len(in_names)
    n_outs = len(out_avals)
    all_in_names = list(in_names) + list(out_names)
    if partition_name is not None:
        all_in_names.append(partition_name)
    donate = tuple(range(n_params, n_params + n_outs))

    def _body(*args):
        operands = list(args)
        if partition_name is not None:
            operands.append(b2j.partition_id_tensor())
        outs = b2j._bass_exec_p.bind(
            *operands, out_avals=tuple(out_avals), in_names=tuple(all_in_names),
            out_names=tuple(out_names), lowering_input_output_aliases=(),
            sim_require_finite=True, sim_require_nnan=True, nc=nc)
        return tuple(outs)

    import jax as _jax
    devices = _jax.devices()[:n_cores]
    mesh = Mesh(np.asarray(devices), ("core",))
    in_specs = (PartitionSpec("core"),) * (n_params + n_outs)
    out_specs = (PartitionSpec("core"),) * n_outs
    sharded = _jax.jit(
        shard_map(_body, mesh=mesh, in_specs=in_specs, out_specs=out_specs,
                  check_rep=False),
        donate_argnums=donate, keep_unused=True)

    def run(in_maps):
        per_core = [[np.asarray(m[name]) for name in in_names] for m in in_maps]
        concat_in = [np.concatenate([per_core[c][i] for c in range(n_cores)],
                                    axis=0) for i in range(n_params)]
        concat_zeros = [np.zeros((n_cores * z.shape[0], *z.shape[1:]), z.dtype)
                        for z in zero_outs]
        out_arrs = sharded(*concat_in, *concat_zeros)
        return [
            {name: np.asarray(out_arrs[i]).reshape(n_cores, *out_avals[i].shape)[c]
             for i, name in enumerate(out_names)}
            for c in range(n_cores)
        ]

    return run


def _static_key(st):
    def freeze(v):
        if isinstance(v, dict):
            return tuple(sorted((k, freeze(x)) for k, x in v.items()))
        if isinstance(v, (list, tuple)):
            return tuple(freeze(x) for x in v)
        return v
    return freeze(st)


def kernel(**inputs) -> np.ndarray:
    static, in_maps = _host_prep(inputs)
    key = _static_key(static)
    if key not in _cache:
        nc = _build_program(static)
        _cache.clear()
        _cache[key] = _make_runner(nc, CORES)
        _cache[("nc", key)] = nc
    run = _cache[key]
    results = run(in_maps)
    out = np.concatenate([r["out"] for r in results], axis=0)
    return np.ascontiguousarray(out, np.float32)
